# revision 11
# baseline (speedup 1.0000x reference)
"""MinGRU Trainium2 kernel (v4).

Problem: nn_MinGRU (B=8, T=4096, D=1024, fp32)
    k  = h @ W_z.T + b_z
    th = h @ W_h.T + b_h
    z = sigmoid(k);  g(x) = max(x + 0.5, sigmoid(x))  (equals the reference's
    piecewise log-space g since the branches cross only at x = 0)
    h[t] = (1 - z[t]) * h[t-1] + z[t] * g(th[t])

Sharding: data-parallel over batch — core i processes sample i ([T, D]).

Measured-rate-driven design (per core):
  - host pre-transposes h to [D, T]: bf16 copy for the th matmul, and an
    fp8(e4m3) copy packed [128, jp, chunk, 2, 512] for DoubleRow k matmuls
    (2 contraction tiles per PE instruction = 2x bf16 throughput). W_z is
    host-scaled by 32 (fp8 range) and the sigmoid activation scale folds
    the 1/32 back out. The th path stays bf16 — fp8 there fails accuracy.
  - e-outer / chunk-inner matmul sweeps keep each weight tile stationary
    across 4 time-chunk PSUM banks (LDWEIGHTS amortized).
  - elementwise fp32 (bf16 GpSimd ops / all-bf16 scans hit slow paths):
    Act: z = sigmoid(k+bz), s = sigmoid(th+bh), (even chunks) a = 1-z via
    sigmoid(-k-bz); Pool: a = 1-z (odd chunks); DVE: g = (th+bh+.5) max s,
    b = z*g (bf16), one [128, 2048] scan per (e, T-half) (fp32 state).
  - scan output fp32 stored straight to a [D, T] HBM tensor; host
    transposes back. Final (e, half) runs chunk-granular to cut the tail.
"""

import contextlib
import numpy as np
import ml_dtypes
import concourse.bass as bass
import concourse.bacc as bacc
import concourse.mybir as mybir
import concourse.tile as tile
from concourse.bass_utils import run_bass_kernel_spmd

F32 = mybir.dt.float32
BF16 = mybir.dt.bfloat16
FP8 = mybir.dt.float8e4
AF = mybir.ActivationFunctionType
OP = mybir.AluOpType
DR = mybir.MatmulPerfMode.DoubleRow

B, T, D = 8, 4096, 1024
NC_CORES = 8
TC = 512                 # time chunk (one fp32 PSUM bank)
NE = D // 128            # 8 e-tiles
ND = D // 128            # 8 d-tiles
NJP = ND // 2            # 4 DoubleRow contraction pairs
NCH = T // TC            # 8 global chunks
TH = 2048                # T-half
CPH = TH // TC           # 4 chunks per half
K_FP8 = True             # fp8 DoubleRow k-path (W_z scaled by 32)
KSC = 1.0 / 32.0 if K_FP8 else 1.0


def build_program():
    nc = bacc.Bacc("TRN2", target_bir_lowering=False, debug=False)
    hT_d = nc.dram_tensor("hT", [D, T], BF16, kind="ExternalInput").ap()
    if K_FP8:
        # packed [p, jp, chunk, r, t] so DR slices are pair-contiguous
        h8_d = nc.dram_tensor("h8", [128, NJP, NCH, 2, TC], FP8,
                              kind="ExternalInput").ap()
        wz_d = nc.dram_tensor("wz", [128, NJP, NE, 2, 128], FP8,
                              kind="ExternalInput").ap()
    else:
        wz_d = nc.dram_tensor("wz", [128, ND, D], BF16,
                              kind="ExternalInput").ap()
    wh_d = nc.dram_tensor("wh", [128, ND, D], BF16, kind="ExternalInput").ap()
    bz_d = nc.dram_tensor("bz", [128, NE], F32, kind="ExternalInput").ap()
    nbz_d = nc.dram_tensor("nbz", [128, NE], F32, kind="ExternalInput").ap()
    bh_d = nc.dram_tensor("bh", [128, NE], F32, kind="ExternalInput").ap()
    bh05_d = nc.dram_tensor("bh05", [128, NE], F32, kind="ExternalInput").ap()
    out_d = nc.dram_tensor("out", [D, T], F32, kind="ExternalOutput").ap()

    with tile.TileContext(nc) as tc, contextlib.ExitStack() as ctx:
        const = ctx.enter_context(tc.tile_pool(name="const", bufs=1))
        mmps = ctx.enter_context(tc.tile_pool(name="mmps", bufs=8,
                                              space="PSUM"))
        zp = ctx.enter_context(tc.tile_pool(name="zp", bufs=3))
        ap_ = ctx.enter_context(tc.tile_pool(name="ap", bufs=2))
        bp = ctx.enter_context(tc.tile_pool(name="bp", bufs=3))
        sp = ctx.enter_context(tc.tile_pool(name="sp", bufs=3))
        gp = ctx.enter_context(tc.tile_pool(name="gp", bufs=3))
        outp = ctx.enter_context(tc.tile_pool(name="outp", bufs=2))

        # ---- constants / whole-tensor SBUF residents ----
        if K_FP8:
            wz_sb = const.tile([128, NJP, NE, 2, 128], FP8)
            h8_sb = const.tile([128, NJP, NCH, 2, TC], FP8)
        else:
            wz_sb = const.tile([128, ND, D], BF16)
        wh_sb = const.tile([128, ND, D], BF16)
        hT_sb = const.tile([128, ND, T], BF16)
        bz_sb = const.tile([128, NE], F32)
        nbz_sb = const.tile([128, NE], F32)
        bh_sb = const.tile([128, NE], F32)
        bh05_sb = const.tile([128, NE], F32)
        lastcol = const.tile([128, NE], F32)
        nc.sync.dma_start(bz_sb, bz_d)
        nc.sync.dma_start(nbz_sb, nbz_d)
        nc.sync.dma_start(bh_sb, bh_d)
        nc.sync.dma_start(bh05_sb, bh05_d)

        def hT_src(d, hh):
            return bass.AP(tensor=hT_d.tensor,
                           offset=hT_d.offset + d * 128 * T + hh * TH,
                           ap=[[T, 128], [1, TH]])

        # DMA order: k-path feed first, medium granularity so transfers
        # spread across the 16 DMA queues.
        if K_FP8:
            for jp in range(NJP):
                nc.sync.dma_start(wz_sb[:, jp, :, :, :], wz_d[:, jp, :, :, :])
                for cg in range(CPH):
                    nc.sync.dma_start(h8_sb[:, jp, cg, :, :],
                                      h8_d[:, jp, cg, :, :])
        else:
            for d in range(ND):
                nc.sync.dma_start(wz_sb[:, d, :], wz_d[:, d, :])
        for d in range(ND):
            nc.sync.dma_start(wh_sb[:, d, :], wh_d[:, d, :])
            nc.sync.dma_start(hT_sb[:, d, 0:TH], hT_src(d, 0))
        for jp in range(NJP if K_FP8 else 0):
            nc.sync.dma_start(h8_sb[:, jp, CPH:NCH, :, :],
                              h8_d[:, jp, CPH:NCH, :, :])
        for d in range(ND):
            nc.sync.dma_start(hT_sb[:, d, TH:T], hT_src(d, 1))

        def k_sweep(hh, e, chunk_outer):
            es = slice(e * 128, (e + 1) * 128)
            kps = [mmps.tile([128, TC], F32, name=f"k{hh}_{e}_{c}", tag="mm")
                   for c in range(CPH)]
            if K_FP8:
                loops = ([(jp, c) for c in range(CPH) for jp in range(NJP)]
                         if chunk_outer else
                         [(jp, c) for jp in range(NJP) for c in range(CPH)])
                for jp, c in loops:
                    cg = hh * CPH + c
                    nc.tensor.matmul(kps[c], wz_sb[:, jp, e, :, :],
                                     h8_sb[:, jp, cg, :, :],
                                     start=(jp == 0), stop=(jp == NJP - 1),
                                     perf_mode=DR)
            else:
                loops = ([(d, c) for c in range(CPH) for d in range(ND)]
                         if chunk_outer else
                         [(d, c) for d in range(ND) for c in range(CPH)])
                for d, c in loops:
                    ts = slice(hh * TH + c * TC, hh * TH + (c + 1) * TC)
                    nc.tensor.matmul(kps[c], wz_sb[:, d, es],
                                     hT_sb[:, d, ts],
                                     start=(d == 0), stop=(d == ND - 1))
            return kps

        def th_sweep(hh, e, chunk_outer):
            es = slice(e * 128, (e + 1) * 128)
            thps = [mmps.tile([128, TC], F32, name=f"t{hh}_{e}_{c}", tag="mm")
                    for c in range(CPH)]
            loops = ([(d, c) for c in range(CPH) for d in range(ND)]
                     if chunk_outer else
                     [(d, c) for d in range(ND) for c in range(CPH)])
            for d, c in loops:
                ts = slice(hh * TH + c * TC, hh * TH + (c + 1) * TC)
                nc.tensor.matmul(thps[c], wh_sb[:, d, es], hT_sb[:, d, ts],
                                 start=(d == 0), stop=(d == ND - 1))
            return thps

        def ew_k(hh, e, c, kps, z_t, a_t):
            cs = slice(c * TC, (c + 1) * TC)
            nc.scalar.activation(z_t[:, cs], kps[c], AF.Sigmoid,
                                 bias=bz_sb[:, e:e + 1], scale=KSC)
            nc.scalar.activation(a_t[:, cs], kps[c], AF.Sigmoid,
                                 bias=nbz_sb[:, e:e + 1], scale=-KSC)

        def ew_th(hh, e, c, thps, z_t, g_t, b_t=None):
            cs = slice(c * TC, (c + 1) * TC)
            s_t = sp.tile([128, TC], BF16, name=f"s{hh}_{e}_{c}", tag="s")
            nc.scalar.activation(s_t, thps[c], AF.Sigmoid,
                                 bias=bh_sb[:, e:e + 1])
            nc.vector.scalar_tensor_tensor(g_t[:, cs], thps[c],
                                           bh05_sb[:, e:e + 1],
                                           s_t, op0=OP.add, op1=OP.max)
            if b_t is not None:
                nc.vector.tensor_tensor(b_t[:, cs], z_t[:, cs], g_t[:, cs],
                                        OP.mult)

        def out_dst(e, t_off, n):
            return bass.AP(tensor=out_d.tensor,
                           offset=out_d.offset + e * 128 * T + t_off,
                           ap=[[T, 128], [1, n]])

        tiles = {}

        def alloc_tiles(hh, e):
            z_t = zp.tile([128, TH], BF16, name=f"z{hh}_{e}", tag="z")
            a_t = ap_.tile([128, TH], F32, name=f"a{hh}_{e}", tag="a")
            g_t = gp.tile([128, TH], BF16, name=f"g{hh}_{e}", tag="g")
            b_t = bp.tile([128, TH], BF16, name=f"b{hh}_{e}", tag="b")
            out_e = outp.tile([128, TH], F32, name=f"o{hh}_{e}", tag="o")
            return z_t, a_t, g_t, b_t, out_e

        def do_k(hh, e):
            last = (hh == 1 and e == NE - 1)
            kps = k_sweep(hh, e, chunk_outer=last)
            tiles[(hh, e)] = (kps, alloc_tiles(hh, e))
            z_t, a_t = tiles[(hh, e)][1][0], tiles[(hh, e)][1][1]
            for c in range(CPH):
                ew_k(hh, e, c, kps, z_t, a_t)

        def do_th(hh, e):
            last = (hh == 1 and e == NE - 1)
            kps, (z_t, a_t, g_t, b_t, out_e) = tiles.pop((hh, e))
            thps = th_sweep(hh, e, chunk_outer=last)
            if not last:
                for c in range(CPH):
                    ew_th(hh, e, c, thps, z_t, g_t)
                nc.vector.tensor_tensor(b_t, z_t, g_t, OP.mult)
                init = 0.0 if hh == 0 else lastcol[:, e:e + 1]
                nc.vector.tensor_tensor_scan(out_e, a_t, b_t, init,
                                             OP.mult, OP.add)
                if hh == 0:
                    nc.scalar.copy(lastcol[:, e:e + 1],
                                   out_e[:, TH - 1:TH])
                nc.sync.dma_start(out_dst(e, hh * TH, TH), out_e)
            else:
                for c in range(CPH):
                    cs = slice(c * TC, (c + 1) * TC)
                    ew_th(hh, e, c, thps, z_t, g_t, b_t=b_t)
                    init = (lastcol[:, e:e + 1] if c == 0
                            else out_e[:, c * TC - 1:c * TC])
                    nc.vector.tensor_tensor_scan(out_e[:, cs],
                                                 a_t[:, cs], b_t[:, cs],
                                                 init, OP.mult, OP.add)
                    nc.sync.dma_start(
                        out_dst(e, hh * TH + c * TC, TC), out_e[:, cs])

        steps = [(hh, e) for hh in range(2) for e in range(NE)]
        do_k(*steps[0])
        for i, (hh, e) in enumerate(steps):
            if i + 1 < len(steps):
                do_k(*steps[i + 1])
            do_th(hh, e)

    nc.compile()
    return nc


_nc_cache = None


def _get_program():
    global _nc_cache
    if _nc_cache is None:
        _nc_cache = build_program()
    return _nc_cache


def _make_in_maps(h_prev_layer, W_z, b_z, W_h, b_h):
    # weights to [d, e] lhsT layout, regrouped [d%128, d_tile, e]
    wzT = np.ascontiguousarray(W_z.T.reshape(ND, 128, D).transpose(1, 0, 2))
    whT = np.ascontiguousarray(W_h.T.reshape(ND, 128, D).transpose(1, 0, 2))
    if K_FP8:
        # [p, dt, e] -> [p, jp, e_t, r, col], pair-contiguous DR layout
        wz = np.ascontiguousarray(
            (wzT * 32.0).astype(ml_dtypes.float8_e4m3)
            .reshape(128, NJP, 2, NE, 128).transpose(0, 1, 3, 2, 4))
    else:
        wz = wzT.astype(ml_dtypes.bfloat16)
    wh = whT.astype(ml_dtypes.bfloat16)
    bz8 = np.ascontiguousarray(b_z.reshape(NE, 128).T.astype(np.float32))
    bh8 = np.ascontiguousarray(b_h.reshape(NE, 128).T.astype(np.float32))
    ins = []
    for i in range(B):
        hT = np.ascontiguousarray(h_prev_layer[i].T)
        m = {"hT": hT.astype(ml_dtypes.bfloat16), "wz": wz, "wh": wh,
             "bz": bz8, "nbz": -bz8, "bh": bh8, "bh05": bh8 + 0.5}
        if K_FP8:
            # [d, t] -> [p, jp, chunk, r, t] pair-contiguous DR layout
            m["h8"] = np.ascontiguousarray(
                hT.astype(ml_dtypes.float8_e4m3)
                .reshape(NJP, 2, 128, NCH, TC).transpose(2, 0, 3, 1, 4))
        ins.append(m)
    return ins


def run(inputs, trace=False, **kw):
    nc = _get_program()
    in_maps = _make_in_maps(**inputs)
    res = run_bass_kernel_spmd(nc, in_maps, core_ids=list(range(NC_CORES)),
                               trace=trace, **kw)
    out = np.stack([res.results[i]["out"].T for i in range(NC_CORES)],
               axis=0).astype(np.float32)
    return np.ascontiguousarray(out), res


def kernel(h_prev_layer, W_z, b_z, W_h, b_h):
    out, _ = run(dict(h_prev_layer=h_prev_layer, W_z=W_z, b_z=b_z,
                      W_h=W_h, b_h=b_h))
    return out


# revision 12
# speedup vs baseline: 1.1003x; 1.1003x over previous
"""MinGRU Trainium2 kernel (v4).

Problem: nn_MinGRU (B=8, T=4096, D=1024, fp32)
    k  = h @ W_z.T + b_z
    th = h @ W_h.T + b_h
    z = sigmoid(k);  g(x) = max(x + 0.5, sigmoid(x))  (equals the reference's
    piecewise log-space g since the branches cross only at x = 0)
    h[t] = (1 - z[t]) * h[t-1] + z[t] * g(th[t])

Sharding: data-parallel over batch — core i processes sample i ([T, D]).

Measured-rate-driven design (per core):
  - host pre-transposes h to [D, T]: bf16 copy for the th matmul, and an
    fp8(e4m3) copy packed [128, jp, chunk, 2, 512] for DoubleRow k matmuls
    (2 contraction tiles per PE instruction = 2x bf16 throughput). W_z is
    host-scaled by 32 (fp8 range) and the sigmoid activation scale folds
    the 1/32 back out. The th path stays bf16 — fp8 there fails accuracy.
  - e-outer / chunk-inner matmul sweeps keep each weight tile stationary
    across 4 time-chunk PSUM banks (LDWEIGHTS amortized).
  - elementwise fp32 (bf16 GpSimd ops / all-bf16 scans hit slow paths):
    Act: z = sigmoid(k+bz), s = sigmoid(th+bh), (even chunks) a = 1-z via
    sigmoid(-k-bz); Pool: a = 1-z (odd chunks); DVE: g = (th+bh+.5) max s,
    b = z*g (bf16), one [128, 2048] scan per (e, T-half) (fp32 state).
  - scan output fp32 stored straight to a [D, T] HBM tensor; host
    transposes back. Final (e, half) runs chunk-granular to cut the tail.
"""

import contextlib
import numpy as np
import ml_dtypes
import concourse.bass as bass
import concourse.bacc as bacc
import concourse.mybir as mybir
import concourse.tile as tile
from concourse.bass_utils import run_bass_kernel_spmd

F32 = mybir.dt.float32
BF16 = mybir.dt.bfloat16
FP8 = mybir.dt.float8e4
AF = mybir.ActivationFunctionType
OP = mybir.AluOpType
DR = mybir.MatmulPerfMode.DoubleRow

B, T, D = 8, 4096, 1024
NC_CORES = 8
TC = 512                 # time chunk (one fp32 PSUM bank)
NE = D // 128            # 8 e-tiles
ND = D // 128            # 8 d-tiles
NJP = ND // 2            # 4 DoubleRow contraction pairs
NCH = T // TC            # 8 global chunks
TH = 2048                # T-half
CPH = TH // TC           # 4 chunks per half
K_FP8 = True             # fp8 DoubleRow k-path (W_z scaled by 32)
KSC = 1.0 / 32.0 if K_FP8 else 1.0


def build_program():
    nc = bacc.Bacc("TRN2", target_bir_lowering=False, debug=False)
    hT_d = nc.dram_tensor("hT", [D, T], BF16, kind="ExternalInput").ap()
    if K_FP8:
        # packed [p, jp, chunk, r, t] so DR slices are pair-contiguous
        h8_d = nc.dram_tensor("h8", [128, NJP, NCH, 2, TC], FP8,
                              kind="ExternalInput").ap()
        wz_d = nc.dram_tensor("wz", [128, NJP, NE, 2, 128], FP8,
                              kind="ExternalInput").ap()
    else:
        wz_d = nc.dram_tensor("wz", [128, ND, D], BF16,
                              kind="ExternalInput").ap()
    wh_d = nc.dram_tensor("wh", [128, ND, D], BF16, kind="ExternalInput").ap()
    bz_d = nc.dram_tensor("bz", [128, NE], F32, kind="ExternalInput").ap()
    nbz_d = nc.dram_tensor("nbz", [128, NE], F32, kind="ExternalInput").ap()
    bh_d = nc.dram_tensor("bh", [128, NE], F32, kind="ExternalInput").ap()
    bh05_d = nc.dram_tensor("bh05", [128, NE], F32, kind="ExternalInput").ap()
    out_d = nc.dram_tensor("out", [D, T], F32, kind="ExternalOutput").ap()

    with tile.TileContext(nc) as tc, contextlib.ExitStack() as ctx:
        const = ctx.enter_context(tc.tile_pool(name="const", bufs=1))
        mmps = ctx.enter_context(tc.tile_pool(name="mmps", bufs=8,
                                              space="PSUM"))
        zp = ctx.enter_context(tc.tile_pool(name="zp", bufs=3))
        ap_ = ctx.enter_context(tc.tile_pool(name="ap", bufs=2))
        bp = ctx.enter_context(tc.tile_pool(name="bp", bufs=3))
        sp = ctx.enter_context(tc.tile_pool(name="sp", bufs=3))
        gp = ctx.enter_context(tc.tile_pool(name="gp", bufs=3))
        outp = ctx.enter_context(tc.tile_pool(name="outp", bufs=2))

        # ---- constants / whole-tensor SBUF residents ----
        if K_FP8:
            wz_sb = const.tile([128, NJP, NE, 2, 128], FP8)
            h8_sb = const.tile([128, NJP, NCH, 2, TC], FP8)
        else:
            wz_sb = const.tile([128, ND, D], BF16)
        wh_sb = const.tile([128, ND, D], BF16)
        hT_sb = const.tile([128, ND, T], BF16)
        bz_sb = const.tile([128, NE], F32)
        nbz_sb = const.tile([128, NE], F32)
        bh_sb = const.tile([128, NE], F32)
        bh05_sb = const.tile([128, NE], F32)
        lastcol = const.tile([128, NE], F32)
        nc.sync.dma_start(bz_sb, bz_d)
        nc.sync.dma_start(nbz_sb, nbz_d)
        nc.sync.dma_start(bh_sb, bh_d)
        nc.sync.dma_start(bh05_sb, bh05_d)

        def hT_src(d, hh):
            return bass.AP(tensor=hT_d.tensor,
                           offset=hT_d.offset + d * 128 * T + hh * TH,
                           ap=[[T, 128], [1, TH]])

        # DMA order: k-path feed first, medium granularity so transfers
        # spread across the 16 DMA queues.
        if K_FP8:
            for jp in range(NJP):
                nc.sync.dma_start(wz_sb[:, jp, :, :, :], wz_d[:, jp, :, :, :])
                for cg in range(CPH):
                    nc.sync.dma_start(h8_sb[:, jp, cg, :, :],
                                      h8_d[:, jp, cg, :, :])
        else:
            for d in range(ND):
                nc.sync.dma_start(wz_sb[:, d, :], wz_d[:, d, :])
        for d in range(ND):
            nc.sync.dma_start(wh_sb[:, d, :], wh_d[:, d, :])
            nc.sync.dma_start(hT_sb[:, d, 0:TH], hT_src(d, 0))
        for jp in range(NJP if K_FP8 else 0):
            nc.sync.dma_start(h8_sb[:, jp, CPH:NCH, :, :],
                              h8_d[:, jp, CPH:NCH, :, :])
        for d in range(ND):
            nc.sync.dma_start(hT_sb[:, d, TH:T], hT_src(d, 1))

        def k_sweep(hh, e, chunk_outer):
            es = slice(e * 128, (e + 1) * 128)
            kps = [mmps.tile([128, TC], F32, name=f"k{hh}_{e}_{c}", tag="mm")
                   for c in range(CPH)]
            if K_FP8:
                loops = ([(jp, c) for c in range(CPH) for jp in range(NJP)]
                         if chunk_outer else
                         [(jp, c) for jp in range(NJP) for c in range(CPH)])
                for jp, c in loops:
                    cg = hh * CPH + c
                    nc.tensor.matmul(kps[c], wz_sb[:, jp, e, :, :],
                                     h8_sb[:, jp, cg, :, :],
                                     start=(jp == 0), stop=(jp == NJP - 1),
                                     perf_mode=DR)
            else:
                loops = ([(d, c) for c in range(CPH) for d in range(ND)]
                         if chunk_outer else
                         [(d, c) for d in range(ND) for c in range(CPH)])
                for d, c in loops:
                    ts = slice(hh * TH + c * TC, hh * TH + (c + 1) * TC)
                    nc.tensor.matmul(kps[c], wz_sb[:, d, es],
                                     hT_sb[:, d, ts],
                                     start=(d == 0), stop=(d == ND - 1))
            return kps

        def th_sweep(hh, e, chunk_outer):
            es = slice(e * 128, (e + 1) * 128)
            thps = [mmps.tile([128, TC], F32, name=f"t{hh}_{e}_{c}", tag="mm")
                    for c in range(CPH)]
            loops = ([(d, c) for c in range(CPH) for d in range(ND)]
                     if chunk_outer else
                     [(d, c) for d in range(ND) for c in range(CPH)])
            for d, c in loops:
                ts = slice(hh * TH + c * TC, hh * TH + (c + 1) * TC)
                nc.tensor.matmul(thps[c], wh_sb[:, d, es], hT_sb[:, d, ts],
                                 start=(d == 0), stop=(d == ND - 1))
            return thps

        def ew_k(hh, e, c, kps, z_t, a_t):
            cs = slice(c * TC, (c + 1) * TC)
            nc.scalar.activation(z_t[:, cs], kps[c], AF.Sigmoid,
                                 bias=bz_sb[:, e:e + 1], scale=KSC)
            nc.scalar.activation(a_t[:, cs], kps[c], AF.Sigmoid,
                                 bias=nbz_sb[:, e:e + 1], scale=-KSC)

        def ew_th(hh, e, c, thps, z_t, g_t, b_t=None):
            cs = slice(c * TC, (c + 1) * TC)
            s_t = sp.tile([128, TC], BF16, name=f"s{hh}_{e}_{c}", tag="s")
            nc.scalar.activation(s_t, thps[c], AF.Sigmoid,
                                 bias=bh_sb[:, e:e + 1])
            nc.vector.scalar_tensor_tensor(g_t[:, cs], thps[c],
                                           bh05_sb[:, e:e + 1],
                                           s_t, op0=OP.add, op1=OP.max)
            if b_t is not None:
                nc.vector.tensor_tensor(b_t[:, cs], z_t[:, cs], g_t[:, cs],
                                        OP.mult)

        def out_dst(e, t_off, n):
            return bass.AP(tensor=out_d.tensor,
                           offset=out_d.offset + e * 128 * T + t_off,
                           ap=[[T, 128], [1, n]])

        tiles = {}

        def alloc_tiles(hh, e):
            z_t = zp.tile([128, TH], BF16, name=f"z{hh}_{e}", tag="z")
            a_t = ap_.tile([128, TH], F32, name=f"a{hh}_{e}", tag="a")
            g_t = gp.tile([128, TH], BF16, name=f"g{hh}_{e}", tag="g")
            b_t = bp.tile([128, TH], BF16, name=f"b{hh}_{e}", tag="b")
            out_e = outp.tile([128, TH], F32, name=f"o{hh}_{e}", tag="o")
            return z_t, a_t, g_t, b_t, out_e

        def do_k(hh, e):
            last = (hh == 1 and e == NE - 1)
            kps = k_sweep(hh, e, chunk_outer=last)
            tiles[(hh, e)] = (kps, alloc_tiles(hh, e))
            z_t, a_t = tiles[(hh, e)][1][0], tiles[(hh, e)][1][1]
            for c in range(CPH):
                ew_k(hh, e, c, kps, z_t, a_t)

        def do_th(hh, e):
            last = (hh == 1 and e == NE - 1)
            kps, (z_t, a_t, g_t, b_t, out_e) = tiles.pop((hh, e))
            thps = th_sweep(hh, e, chunk_outer=last)
            if not last:
                for c in range(CPH):
                    ew_th(hh, e, c, thps, z_t, g_t)
                nc.vector.tensor_tensor(b_t, z_t, g_t, OP.mult)
                init = 0.0 if hh == 0 else lastcol[:, e:e + 1]
                nc.vector.tensor_tensor_scan(out_e, a_t, b_t, init,
                                             OP.mult, OP.add)
                if hh == 0:
                    nc.scalar.copy(lastcol[:, e:e + 1],
                                   out_e[:, TH - 1:TH])
                nc.sync.dma_start(out_dst(e, hh * TH, TH), out_e)
            else:
                for c in range(CPH):
                    cs = slice(c * TC, (c + 1) * TC)
                    ew_th(hh, e, c, thps, z_t, g_t, b_t=b_t)
                    init = (lastcol[:, e:e + 1] if c == 0
                            else out_e[:, c * TC - 1:c * TC])
                    nc.vector.tensor_tensor_scan(out_e[:, cs],
                                                 a_t[:, cs], b_t[:, cs],
                                                 init, OP.mult, OP.add)
                    nc.sync.dma_start(
                        out_dst(e, hh * TH + c * TC, TC), out_e[:, cs])

        for hh in range(2):
            for e in range(NE):
                do_k(hh, e)
                do_th(hh, e)

    nc.compile()
    return nc


_nc_cache = None


def _get_program():
    global _nc_cache
    if _nc_cache is None:
        _nc_cache = build_program()
    return _nc_cache


def _make_in_maps(h_prev_layer, W_z, b_z, W_h, b_h):
    # weights to [d, e] lhsT layout, regrouped [d%128, d_tile, e]
    wzT = np.ascontiguousarray(W_z.T.reshape(ND, 128, D).transpose(1, 0, 2))
    whT = np.ascontiguousarray(W_h.T.reshape(ND, 128, D).transpose(1, 0, 2))
    if K_FP8:
        # [p, dt, e] -> [p, jp, e_t, r, col], pair-contiguous DR layout
        wz = np.ascontiguousarray(
            (wzT * 32.0).astype(ml_dtypes.float8_e4m3)
            .reshape(128, NJP, 2, NE, 128).transpose(0, 1, 3, 2, 4))
    else:
        wz = wzT.astype(ml_dtypes.bfloat16)
    wh = whT.astype(ml_dtypes.bfloat16)
    bz8 = np.ascontiguousarray(b_z.reshape(NE, 128).T.astype(np.float32))
    bh8 = np.ascontiguousarray(b_h.reshape(NE, 128).T.astype(np.float32))
    ins = []
    for i in range(B):
        hT = np.ascontiguousarray(h_prev_layer[i].T)
        m = {"hT": hT.astype(ml_dtypes.bfloat16), "wz": wz, "wh": wh,
             "bz": bz8, "nbz": -bz8, "bh": bh8, "bh05": bh8 + 0.5}
        if K_FP8:
            # [d, t] -> [p, jp, chunk, r, t] pair-contiguous DR layout
            m["h8"] = np.ascontiguousarray(
                hT.astype(ml_dtypes.float8_e4m3)
                .reshape(NJP, 2, 128, NCH, TC).transpose(2, 0, 3, 1, 4))
        ins.append(m)
    return ins


def run(inputs, trace=False, **kw):
    nc = _get_program()
    in_maps = _make_in_maps(**inputs)
    res = run_bass_kernel_spmd(nc, in_maps, core_ids=list(range(NC_CORES)),
                               trace=trace, **kw)
    out = np.stack([res.results[i]["out"].T for i in range(NC_CORES)],
               axis=0).astype(np.float32)
    return np.ascontiguousarray(out), res


def kernel(h_prev_layer, W_z, b_z, W_h, b_h):
    out, _ = run(dict(h_prev_layer=h_prev_layer, W_z=W_z, b_z=b_z,
                      W_h=W_h, b_h=b_h))
    return out
